# revision 12
# baseline (speedup 1.0000x reference)
"""Trainium2 Bass kernel for nn_DglAggregator (GNN message passing).

Strategy (8 NeuronCores, SPMD, one uniform program, per-core data):
- Targets are partitioned across cores balanced by stage-1 edge count; each
  core owns its targets' items and ALL stage-1 edges pointing at those items,
  so no cross-core communication is needed.
- Stage 1 (item->item segment softmax + weighted sum): edges sorted by dst
  item; per 1024-edge window the per-edge weights exp(score) are folded into
  a [128 edges x 128 slots] selection matrix and accumulated into per-window
  PSUM via TensorE matmuls (unnormalized sums + denominators); normalization
  is a per-slot row scale at readout. Softmax max-subtraction is skipped:
  scores are O(1) here (|score| < ~6) so exp is exact-safe in f32.
- The x_src rows are fetched with the int16 dma_gather ucode. The global
  table (200k rows) exceeds int16, so gathers are issued per 32768-row chunk
  (edges of each batch grouped by chunk, quota-padded), staged through a DRAM
  scratch, and re-gathered (int16 positions) into dst-sorted order.
- The x_dst rows come from a per-core local table (item-slot order, <=32k
  rows -> single int16 gather), pre-scaled by pi on device.
- Stage 2 (item->target): mean/deg, f = [h_t, mean] @ r_w, e2 = tanh([ft,
  h_p] @ q_w), w = <e2, f[dst]>, out = sum w*ft — all via the same masked
  matmul pattern over 128-target windows; per-edge f rows via int16 gather
  from an on-device f table.
- Host-side work is limited to graph restructuring: integer index math and
  row permutations of input tables (sharding). All floating-point arithmetic
  on the data path runs on the NeuronCores.

kernel(**inputs) accepts the FULL unsharded inputs and returns the FULL
[N_TGT, 128] output.
"""
import numpy as np

P = 128          # partitions / tile edge
D = 128          # feature dim
NCORES = 8
CHUNK = 32768    # int16-addressable table chunk
WE1 = 1024       # stage-1 window edge capacity (8 tiles)
WS1 = 128        # stage-1 window slot capacity
WB = 8           # stage-1 windows per batch
TI2 = 50         # stage-2 tiles per window (6400 item slots)
WS2 = 128        # stage-2 window target capacity
GH = 4096        # gather granularity for g2/xd (half batch)
BUFS = {"xs": 2, "gat": 2, "wk": 3, "sm": 4, "pp": 2, "ip1": 2, "p0": 3,
        "bg": 2, "wk2": 3, "ppB": 2}
PHASES = 2       # debug: 0 = P0 only, 1 = +stage-1, 2 = full
_LAST_NC = None
P1SUB = "full"   # debug: "gather" | "dve" | "full"
ABL = set()      # timing ablations: g1 scr g2 xd scores dvemask mm


def _wrap_idx16(idx: np.ndarray, cap: int) -> np.ndarray:
    """[n<=cap] -> [128, cap/16] int16 (j at [j%16, j//16], replicated x8).
    Pad with 0 (row 0 is always a valid gather target)."""
    a = np.zeros(cap, np.int64)
    a[: idx.shape[0]] = idx
    assert cap % 16 == 0
    assert a.min() >= 0 and a.max() < 32768, (a.min(), a.max())
    blk = a.reshape(cap // 16, 16).T.astype(np.int16)
    return np.tile(blk, (8, 1))


def _interleave_f32(vals: np.ndarray, cap: int, fill: float) -> np.ndarray:
    """[n] -> [128, cap/128] f32 with value of rank r at [r%128, r//128]."""
    a = np.full(cap, fill, np.float32)
    a[: vals.shape[0]] = vals
    return a.reshape(cap // P, P).T.copy()


def _pack_runs(run_sizes, max_runs, max_total):
    """Greedy pack consecutive runs into groups: each group holds whole runs,
    <= max_runs runs and <= max_total total size. Oversized single runs are
    rejected (assert). Returns list of (start_run, n_runs)."""
    groups = []
    i, n = 0, len(run_sizes)
    while i < n:
        tot, j = 0, i
        while j < n and j - i < max_runs and tot + run_sizes[j] <= max_total:
            tot += run_sizes[j]
            j += 1
        assert j > i, f"run {i} of size {run_sizes[i]} exceeds {max_total}"
        groups.append((i, j - i))
        i = j
    return groups


def preprocess(h_v, h_p, h_t, int_src, int_dst, agg_dst):
    """All graph restructuring. Returns shared dims + per-core arrays."""
    NITEM = h_v.shape[0]
    NTGT = h_t.shape[0]
    int_src = int_src.astype(np.int64)
    int_dst = int_dst.astype(np.int64)
    item_tgt = agg_dst.astype(np.int64)          # item i -> target (agg_src=arange)
    n_chunks = (NITEM + CHUNK - 1) // CHUNK

    # ---- target -> core, balanced by stage-1 edge load ----
    deg_int = np.bincount(int_dst, minlength=NITEM)
    t_edges = np.bincount(item_tgt, weights=deg_int.astype(np.float64),
                          minlength=NTGT)
    t_items = np.bincount(item_tgt, minlength=NTGT)
    tgt_core = np.zeros(NTGT, np.int64)
    load = np.zeros(NCORES)
    for t in np.argsort(-t_edges, kind="stable"):
        c = int(np.argmin(load))
        tgt_core[t] = c
        load[c] += t_edges[t] + 0.5 * t_items[t]

    item_core = tgt_core[item_tgt]

    cores = []
    for c in range(NCORES):
        tlist = np.where(tgt_core == c)[0]
        items = np.where(item_core == c)[0]
        # items ordered by (target, item id)
        items = items[np.lexsort((items, item_tgt[items]))]
        cores.append({"targets": tlist, "items": items})

    # ---- stage-2 windows (whole targets, <=WS2 targets, <=TI2*128 islots) ----
    for c in range(NCORES):
        st = cores[c]
        tl = st["targets"]
        sizes = t_items[tl]
        groups = _pack_runs(sizes, WS2, TI2 * P)
        st["w2groups"] = groups
    W2 = max(len(st["w2groups"]) for st in cores)
    NI = W2 * TI2 * P

    for c in range(NCORES):
        st = cores[c]
        tl, items = st["targets"], st["items"]
        it_item = np.full(NI, -1, np.int64)       # islot -> global item
        it_tgtloc = np.full(NI, -1.0, np.float32)  # islot -> window-local tgt
        it_tslot = np.zeros(NI, np.int64)          # islot -> global tgt slot
        twin = np.full((W2, WS2), -1, np.int64)    # window -> global targets
        ipos = 0  # position within items array
        for w2, (t0, ntgt) in enumerate(st["w2groups"]):
            base = w2 * TI2 * P
            off = 0
            for k in range(ntgt):
                t = tl[t0 + k]
                cnt = int(t_items[t])
                sl = slice(base + off, base + off + cnt)
                it_item[sl] = items[ipos : ipos + cnt]
                it_tgtloc[sl] = k
                it_tslot[sl] = w2 * WS2 + k
                twin[w2, k] = t
                ipos += cnt
                off += cnt
        assert ipos == len(items)
        st["it_item"] = it_item
        st["it_tgtloc"] = it_tgtloc
        st["it_tslot"] = it_tslot
        st["twin"] = twin
        islot_of = np.full(NITEM, -1, np.int64)
        real = it_item >= 0
        islot_of[it_item[real]] = np.where(real)[0]
        st["islot_of"] = islot_of

    # ---- stage-1 edges, windows ----
    for c in range(NCORES):
        st = cores[c]
        emask = item_core[int_dst] == c
        es = int_src[emask]
        ed = st["islot_of"][int_dst[emask]]
        o = np.argsort(ed, kind="stable")
        es, ed = es[o], ed[o]
        # windows over whole dst-slot runs
        uslots, ustart, ucnt = np.unique(ed, return_index=True, return_counts=True)
        groups = _pack_runs(ucnt, WS1, WE1)
        st["e_src"] = es
        st["e_dst"] = ed
        st["w1groups"] = groups
        st["uslots"] = uslots
        st["ustart"] = ustart
        st["ucnt"] = ucnt
    W1 = max(len(st["w1groups"]) for st in cores)
    W1 = ((W1 + WB - 1) // WB) * WB
    B1 = W1 // WB
    assert W1 * WS1 + P <= 32768, f"FT table too big for int16: W1={W1}"

    # per-window edge/seg arrays (original w1groups order)
    for c in range(NCORES):
        st = cores[c]
        es, ed = st["e_src"], st["e_dst"]
        uslots, ustart, ucnt = st["uslots"], st["ustart"], st["ucnt"]
        wsrc = np.zeros((W1, WE1), np.int64)       # src ids (pad 0)
        wdst = np.zeros((W1, WE1), np.int64)       # dst islot (pad 0)
        wseg = np.full((W1, WE1), -1.0, np.float32)  # window-local slot (pad -1)
        wcnt = np.zeros(W1, np.int64)
        for w, (r0, nr) in enumerate(st["w1groups"]):
            e0 = ustart[r0]
            ne = int(ucnt[r0 : r0 + nr].sum())
            wsrc[w, :ne] = es[e0 : e0 + ne]
            wdst[w, :ne] = ed[e0 : e0 + ne]
            lab = np.repeat(np.arange(nr), ucnt[r0 : r0 + nr])
            wseg[w, :ne] = lab
            wcnt[w] = ne
        st["wsrc"], st["wdst"], st["wseg"], st["wcnt"] = wsrc, wdst, wseg, wcnt

    # ---- batches: assign windows to batches balancing chunk quotas, then
    # renumber windows by (batch, rank-in-batch) so FT readout offsets are
    # uniform code across cores ----
    for c in range(NCORES):
        st = cores[c]
        wsrc, wcnt = st["wsrc"], st["wcnt"]
        ck = wsrc // CHUNK
        ck[np.arange(WE1)[None, :] >= wcnt[:, None]] = 0  # pads -> chunk 0
        cnts = np.zeros((W1, n_chunks), np.int64)
        for k in range(n_chunks):
            cnts[:, k] = (ck == k).sum(1)
        # pads counted in chunk 0 — treat as real work for quota purposes
        order = np.argsort(-cnts.max(1), kind="stable")
        bload = np.zeros((B1, n_chunks), np.int64)
        bfill = np.zeros(B1, np.int64)
        wbatch = np.zeros(W1, np.int64)
        for w in order:
            cand = np.where(bfill < WB)[0]
            j = cand[np.argmin((bload[cand] + cnts[w]).max(1))]
            wbatch[w] = j
            bload[j] += cnts[w]
            bfill[j] += 1
        st["bload"] = bload
        # new order: stable sort by batch; window new index = position
        neww = np.argsort(wbatch, kind="stable")   # new_idx -> old_idx
        st["wsrc"] = st["wsrc"][neww]
        st["wdst"] = st["wdst"][neww]
        st["wseg"] = st["wseg"][neww]
        st["wcnt"] = st["wcnt"][neww]
        # ft slots follow the NEW window numbering
        ft_slot = np.full(NI, W1 * WS1, np.int64)  # default: zero page
        old2new = np.argsort(neww, kind="stable")
        uslots = st["uslots"]
        for wold, (r0, nr) in enumerate(st["w1groups"]):
            wnew = old2new[wold]
            ft_slot[uslots[r0 : r0 + nr]] = wnew * WS1 + np.arange(nr)
        st["ft_slot"] = ft_slot
    # shared quotas (multiple of 128, >=128)
    Q = np.zeros(n_chunks, np.int64)
    for c in range(NCORES):
        Q = np.maximum(Q, cores[c]["bload"].max(0))
    # 256-multiples keep every idx-tile slice offset 32B-aligned for the
    # gather ucode
    Q = np.maximum(((Q + 255) // 256) * 256, 256)
    SC = int(Q.sum())
    Qoff = np.concatenate([[0], np.cumsum(Q)])

    # ---- per-batch gather arrays (batch b = windows [b*WB, (b+1)*WB)) ----
    for c in range(NCORES):
        st = cores[c]
        g1 = np.zeros((B1, P, SC // 16), np.int16)
        g2 = np.zeros((B1, P, (WB * WE1) // 16), np.int16)
        xd = np.zeros((B1, P, (WB * WE1) // 16), np.int16)
        seg = np.full((B1, P, (WB * WE1) // P), -1.0, np.float32)
        for b in range(B1):
            wins = np.arange(b * WB, (b + 1) * WB)
            src = st["wsrc"][wins].reshape(-1)        # [WB*WE1] rank order
            dst = st["wdst"][wins].reshape(-1)
            sg = st["wseg"][wins].reshape(-1)
            ck = src // CHUNK
            pos = np.zeros(WB * WE1, np.int64)
            g1i = np.zeros(SC, np.int64)
            for k in range(n_chunks):
                sel = np.where(ck == k)[0]
                assert len(sel) <= Q[k], (c, b, k, len(sel), Q[k])
                pos[sel] = Qoff[k] + np.arange(len(sel))
                g1i[Qoff[k] : Qoff[k] + len(sel)] = src[sel] - k * CHUNK
            g1[b] = _wrap_idx16(g1i, SC)
            g2[b] = _wrap_idx16(pos, WB * WE1)
            xd[b] = _wrap_idx16(dst, WB * WE1)
            seg[b] = _interleave_f32(sg, WB * WE1, -1.0)
        st["g1"], st["g2"], st["xd"], st["seg"] = g1, g2, xd, seg

    # ---- stage-2 gather/meta arrays + tables ----
    for c in range(NCORES):
        st = cores[c]
        it_item = st["it_item"]
        real = it_item >= 0
        st["ftg"] = _wrap_idx16(st["ft_slot"], NI)
        st["fexp"] = _wrap_idx16(st["it_tslot"], NI)
        tl = np.zeros((W2, P, TI2), np.float32)
        for w2 in range(W2):
            tl[w2] = _interleave_f32(
                st["it_tgtloc"][w2 * TI2 * P : (w2 + 1) * TI2 * P], TI2 * P, -1.0
            )
        st["tgtloc"] = tl
        hv_local = np.zeros((NI, D), np.float32)
        hv_local[real] = h_v[it_item[real]]
        st["hv_local"] = hv_local
        hpT = np.zeros((D, NI), np.float32)
        hpT[:, real] = h_p[it_item[real]].T
        st["hpT"] = hpT
        htT = np.zeros((D, W2 * WS2), np.float32)
        tw = st["twin"].reshape(-1)
        htT[:, tw >= 0] = h_t[tw[tw >= 0]].T
        st["htT"] = htT

    dims = {
        "NI": NI, "W1": W1, "B1": B1, "W2": W2, "SC": SC,
        "Q": Q.tolist(), "Qoff": Qoff.tolist(), "n_chunks": n_chunks,
        "NITEM": NITEM, "NTGT": NTGT,
    }
    return dims, cores


# revision 14
# speedup vs baseline: 1.0285x; 1.0285x over previous
"""Trainium2 Bass kernel for nn_DglAggregator (GNN message passing).

Strategy (8 NeuronCores, SPMD, one uniform program, per-core data):
- Targets are partitioned across cores balanced by stage-1 edge count; each
  core owns its targets' items and ALL stage-1 edges pointing at those items,
  so no cross-core communication is needed.
- Stage 1 (item->item segment softmax + weighted sum): edges sorted by dst
  item; per 1024-edge window the per-edge weights exp(score) are folded into
  a [128 edges x 128 slots] selection matrix and accumulated into per-window
  PSUM via TensorE matmuls (unnormalized sums + denominators); normalization
  is a per-slot row scale at readout. Softmax max-subtraction is skipped:
  scores are O(1) here (|score| < ~6) so exp is exact-safe in f32.
- The x_src rows are fetched with the int16 dma_gather ucode. The global
  table (200k rows) exceeds int16, so gathers are issued per 32768-row chunk
  (edges of each batch grouped by chunk, quota-padded), staged through a DRAM
  scratch, and re-gathered (int16 positions) into dst-sorted order.
- The x_dst rows come from a per-core local table (item-slot order, <=32k
  rows -> single int16 gather), pre-scaled by pi on device.
- Stage 2 (item->target): mean/deg, f = [h_t, mean] @ r_w, e2 = tanh([ft,
  h_p] @ q_w), w = <e2, f[dst]>, out = sum w*ft — all via the same masked
  matmul pattern over 128-target windows; per-edge f rows via int16 gather
  from an on-device f table.
- Host-side work is limited to graph restructuring: integer index math and
  row permutations of input tables (sharding). All floating-point arithmetic
  on the data path runs on the NeuronCores.

kernel(**inputs) accepts the FULL unsharded inputs and returns the FULL
[N_TGT, 128] output.
"""
import numpy as np

P = 128          # partitions / tile edge
D = 128          # feature dim
NCORES = 8
CHUNK = 32768    # int16-addressable table chunk
WE1 = 1024       # stage-1 window edge capacity (8 tiles)
WS1 = 128        # stage-1 window slot capacity
WB = 8           # stage-1 windows per batch
TI2 = 50         # stage-2 tiles per window (6400 item slots)
WS2 = 128        # stage-2 window target capacity
GH = 4096        # gather granularity for g2/xd (half batch)
BUFS = {"xs": 2, "gat": 2, "wk": 3, "sm": 4, "pp": 2, "ip1": 2, "p0": 3,
        "bg": 2, "wk2": 3, "ppB": 2}
PHASES = 2       # debug: 0 = P0 only, 1 = +stage-1, 2 = full
_LAST_NC = None
P1SUB = "full"   # debug: "gather" | "dve" | "full"
ABL = set()      # timing ablations: g1 scr g2 xd scores dvemask mm


def _wrap_idx16(idx: np.ndarray, cap: int) -> np.ndarray:
    """[n<=cap] -> [128, cap/16] int16 (j at [j%16, j//16], replicated x8).
    Pad with 0 (row 0 is always a valid gather target)."""
    a = np.zeros(cap, np.int64)
    a[: idx.shape[0]] = idx
    assert cap % 16 == 0
    assert a.min() >= 0 and a.max() < 32768, (a.min(), a.max())
    blk = a.reshape(cap // 16, 16).T.astype(np.int16)
    return np.tile(blk, (8, 1))


def _interleave_f32(vals: np.ndarray, cap: int, fill: float) -> np.ndarray:
    """[n] -> [128, cap/128] f32 with value of rank r at [r%128, r//128]."""
    a = np.full(cap, fill, np.float32)
    a[: vals.shape[0]] = vals
    return a.reshape(cap // P, P).T.copy()


def _pack_runs(run_sizes, max_runs, max_total):
    """Greedy pack consecutive runs into groups: each group holds whole runs,
    <= max_runs runs and <= max_total total size. Oversized single runs are
    rejected (assert). Returns list of (start_run, n_runs)."""
    groups = []
    i, n = 0, len(run_sizes)
    while i < n:
        tot, j = 0, i
        while j < n and j - i < max_runs and tot + run_sizes[j] <= max_total:
            tot += run_sizes[j]
            j += 1
        assert j > i, f"run {i} of size {run_sizes[i]} exceeds {max_total}"
        groups.append((i, j - i))
        i = j
    return groups


def preprocess(h_v, h_p, h_t, int_src, int_dst, agg_dst):
    """All graph restructuring. Returns shared dims + per-core arrays."""
    NITEM = h_v.shape[0]
    NTGT = h_t.shape[0]
    int_src = int_src.astype(np.int64)
    int_dst = int_dst.astype(np.int64)
    item_tgt = agg_dst.astype(np.int64)          # item i -> target (agg_src=arange)
    n_chunks = (NITEM + CHUNK - 1) // CHUNK

    # ---- target -> core, balanced by stage-1 edge load ----
    deg_int = np.bincount(int_dst, minlength=NITEM)
    t_edges = np.bincount(item_tgt, weights=deg_int.astype(np.float64),
                          minlength=NTGT)
    t_items = np.bincount(item_tgt, minlength=NTGT)
    tgt_core = np.zeros(NTGT, np.int64)
    load = np.zeros(NCORES)
    for t in np.argsort(-t_edges, kind="stable"):
        c = int(np.argmin(load))
        tgt_core[t] = c
        load[c] += t_edges[t] + 0.5 * t_items[t]

    item_core = tgt_core[item_tgt]

    cores = []
    for c in range(NCORES):
        tlist = np.where(tgt_core == c)[0]
        items = np.where(item_core == c)[0]
        # items ordered by (target, item id)
        items = items[np.lexsort((items, item_tgt[items]))]
        cores.append({"targets": tlist, "items": items})

    # ---- stage-2 windows (whole targets, <=WS2 targets, <=TI2*128 islots) ----
    for c in range(NCORES):
        st = cores[c]
        tl = st["targets"]
        sizes = t_items[tl]
        groups = _pack_runs(sizes, WS2, TI2 * P)
        st["w2groups"] = groups
    W2 = max(len(st["w2groups"]) for st in cores)
    NI = W2 * TI2 * P

    for c in range(NCORES):
        st = cores[c]
        tl, items = st["targets"], st["items"]
        it_item = np.full(NI, -1, np.int64)       # islot -> global item
        it_tgtloc = np.full(NI, -1.0, np.float32)  # islot -> window-local tgt
        it_tslot = np.zeros(NI, np.int64)          # islot -> global tgt slot
        twin = np.full((W2, WS2), -1, np.int64)    # window -> global targets
        ipos = 0  # position within items array
        for w2, (t0, ntgt) in enumerate(st["w2groups"]):
            base = w2 * TI2 * P
            off = 0
            for k in range(ntgt):
                t = tl[t0 + k]
                cnt = int(t_items[t])
                sl = slice(base + off, base + off + cnt)
                it_item[sl] = items[ipos : ipos + cnt]
                it_tgtloc[sl] = k
                it_tslot[sl] = w2 * WS2 + k
                twin[w2, k] = t
                ipos += cnt
                off += cnt
        assert ipos == len(items)
        st["it_item"] = it_item
        st["it_tgtloc"] = it_tgtloc
        st["it_tslot"] = it_tslot
        st["twin"] = twin
        islot_of = np.full(NITEM, -1, np.int64)
        real = it_item >= 0
        islot_of[it_item[real]] = np.where(real)[0]
        st["islot_of"] = islot_of

    # ---- stage-1 edges, windows ----
    for c in range(NCORES):
        st = cores[c]
        emask = item_core[int_dst] == c
        es = int_src[emask]
        ed = st["islot_of"][int_dst[emask]]
        o = np.argsort(ed, kind="stable")
        es, ed = es[o], ed[o]
        # windows over whole dst-slot runs
        uslots, ustart, ucnt = np.unique(ed, return_index=True, return_counts=True)
        groups = _pack_runs(ucnt, WS1, WE1)
        st["e_src"] = es
        st["e_dst"] = ed
        st["w1groups"] = groups
        st["uslots"] = uslots
        st["ustart"] = ustart
        st["ucnt"] = ucnt
    W1 = max(len(st["w1groups"]) for st in cores)
    W1 = ((W1 + WB - 1) // WB) * WB
    B1 = W1 // WB
    assert W1 * WS1 + P <= 32768, f"FT table too big for int16: W1={W1}"

    # per-window edge/seg arrays (original w1groups order)
    for c in range(NCORES):
        st = cores[c]
        es, ed = st["e_src"], st["e_dst"]
        uslots, ustart, ucnt = st["uslots"], st["ustart"], st["ucnt"]
        # pad-edge sources spread round-robin across chunks so quota
        # padding doesn't concentrate in chunk 0 (pad rows are gathered but
        # never consumed: their positions are masked via seg=-1)
        nspread = max(1, n_chunks - 1)
        wsrc = (np.arange(WE1, dtype=np.int64)[None, :] % nspread) * CHUNK             + np.zeros((W1, 1), np.int64)
        wdst = np.zeros((W1, WE1), np.int64)       # dst islot (pad 0)
        wseg = np.full((W1, WE1), -1.0, np.float32)  # window-local slot (pad -1)
        wcnt = np.zeros(W1, np.int64)
        for w, (r0, nr) in enumerate(st["w1groups"]):
            e0 = ustart[r0]
            ne = int(ucnt[r0 : r0 + nr].sum())
            wsrc[w, :ne] = es[e0 : e0 + ne]
            wdst[w, :ne] = ed[e0 : e0 + ne]
            lab = np.repeat(np.arange(nr), ucnt[r0 : r0 + nr])
            wseg[w, :ne] = lab
            wcnt[w] = ne
        st["wsrc"], st["wdst"], st["wseg"], st["wcnt"] = wsrc, wdst, wseg, wcnt

    # ---- batches: assign windows to batches balancing chunk quotas, then
    # renumber windows by (batch, rank-in-batch) so FT readout offsets are
    # uniform code across cores ----
    for c in range(NCORES):
        st = cores[c]
        wsrc, wcnt = st["wsrc"], st["wcnt"]
        ck = wsrc // CHUNK
        cnts = np.zeros((W1, n_chunks), np.int64)
        for k in range(n_chunks):
            cnts[:, k] = (ck == k).sum(1)
        # pads counted in chunk 0 — treat as real work for quota purposes
        order = np.argsort(-cnts.max(1), kind="stable")
        bload = np.zeros((B1, n_chunks), np.int64)
        bfill = np.zeros(B1, np.int64)
        wbatch = np.zeros(W1, np.int64)
        for w in order:
            cand = np.where(bfill < WB)[0]
            j = cand[np.argmin((bload[cand] + cnts[w]).max(1))]
            wbatch[w] = j
            bload[j] += cnts[w]
            bfill[j] += 1
        st["bload"] = bload
        # new order: stable sort by batch; window new index = position
        neww = np.argsort(wbatch, kind="stable")   # new_idx -> old_idx
        st["wsrc"] = st["wsrc"][neww]
        st["wdst"] = st["wdst"][neww]
        st["wseg"] = st["wseg"][neww]
        st["wcnt"] = st["wcnt"][neww]
        # ft slots follow the NEW window numbering
        ft_slot = np.full(NI, W1 * WS1, np.int64)  # default: zero page
        old2new = np.argsort(neww, kind="stable")
        uslots = st["uslots"]
        for wold, (r0, nr) in enumerate(st["w1groups"]):
            wnew = old2new[wold]
            ft_slot[uslots[r0 : r0 + nr]] = wnew * WS1 + np.arange(nr)
        st["ft_slot"] = ft_slot
    # shared quotas (multiple of 128, >=128)
    Q = np.zeros(n_chunks, np.int64)
    for c in range(NCORES):
        Q = np.maximum(Q, cores[c]["bload"].max(0))
    # 256-multiples keep every idx-tile slice offset 32B-aligned for the
    # gather ucode
    Q = np.maximum(((Q + 255) // 256) * 256, 256)
    SC = int(Q.sum())
    Qoff = np.concatenate([[0], np.cumsum(Q)])

    # ---- per-batch gather arrays (batch b = windows [b*WB, (b+1)*WB)) ----
    for c in range(NCORES):
        st = cores[c]
        g1 = np.zeros((B1, P, SC // 16), np.int16)
        g2 = np.zeros((B1, P, (WB * WE1) // 16), np.int16)
        xd = np.zeros((B1, P, (WB * WE1) // 16), np.int16)
        seg = np.full((B1, P, (WB * WE1) // P), -1.0, np.float32)
        for b in range(B1):
            wins = np.arange(b * WB, (b + 1) * WB)
            src = st["wsrc"][wins].reshape(-1)        # [WB*WE1] rank order
            dst = st["wdst"][wins].reshape(-1)
            sg = st["wseg"][wins].reshape(-1)
            ck = src // CHUNK
            pos = np.zeros(WB * WE1, np.int64)
            g1i = np.zeros(SC, np.int64)
            for k in range(n_chunks):
                sel = np.where(ck == k)[0]
                assert len(sel) <= Q[k], (c, b, k, len(sel), Q[k])
                pos[sel] = Qoff[k] + np.arange(len(sel))
                g1i[Qoff[k] : Qoff[k] + len(sel)] = src[sel] - k * CHUNK
            g1[b] = _wrap_idx16(g1i, SC)
            g2[b] = _wrap_idx16(pos, WB * WE1)
            xd[b] = _wrap_idx16(dst, WB * WE1)
            seg[b] = _interleave_f32(sg, WB * WE1, -1.0)
        st["g1"], st["g2"], st["xd"], st["seg"] = g1, g2, xd, seg

    # ---- stage-2 gather/meta arrays + tables ----
    for c in range(NCORES):
        st = cores[c]
        it_item = st["it_item"]
        real = it_item >= 0
        st["ftg"] = _wrap_idx16(st["ft_slot"], NI)
        st["fexp"] = _wrap_idx16(st["it_tslot"], NI)
        tl = np.zeros((W2, P, TI2), np.float32)
        for w2 in range(W2):
            tl[w2] = _interleave_f32(
                st["it_tgtloc"][w2 * TI2 * P : (w2 + 1) * TI2 * P], TI2 * P, -1.0
            )
        st["tgtloc"] = tl
        hv_local = np.zeros((NI, D), np.float32)
        hv_local[real] = h_v[it_item[real]]
        st["hv_local"] = hv_local
        hpT = np.zeros((D, NI), np.float32)
        hpT[:, real] = h_p[it_item[real]].T
        st["hpT"] = hpT
        htT = np.zeros((D, W2 * WS2), np.float32)
        tw = st["twin"].reshape(-1)
        htT[:, tw >= 0] = h_t[tw[tw >= 0]].T
        st["htT"] = htT

    dims = {
        "NI": NI, "W1": W1, "B1": B1, "W2": W2, "SC": SC,
        "Q": Q.tolist(), "Qoff": Qoff.tolist(), "n_chunks": n_chunks,
        "NITEM": NITEM, "NTGT": NTGT,
    }
    return dims, cores
